# revision 7
# baseline (speedup 1.0000x reference)
"""Trainium2 Bass kernel for nn_Agent_Aggregator_with_Mask_Denoise_Mechanism.

Sharding: tensor-parallel over heads h (8 heads -> 8 cores). Each core computes
its head end-to-end; the only cross-core value is the scalar threshold logit
(an 8-way AllReduce of a 4-byte partial sum, padded to 64 B). Host does the
final (h d) concat + transpose.

Math notes vs the reference:
- sigmoid(m) > sigmoid(t)  <=>  m > t: the mask threshold compares pre-sigmoid
  logits, so no sigmoid tables are needed (Exp only).
- The reference's kv.reshape(b, a, h*d) row-major remap means the thresh
  weight applied to kv[b,h,a,d] is w_thresh[0, (a%8)*64+d], independent of h.
- q is never materialized: qa logits = x @ (wq^T agent^T scale), with the
  folded matrix precomputed on host, prescaled by 64 and quantized to fp8;
  the matmul runs in DoubleRow mode (2x) and exp(logit/64) undoes the scale.
- k/v projections stay f16 (fp8 there breaks the 2e-2 gate) but share one
  128-wide stationary [v|k], so the PE array is fully used.
- ka logits run as concurrent row-tile pairs: even n-chunks' k lives at
  partitions 64:128, odd chunks' k is DMA-shifted to partitions 0:64, and the
  (64,0)/(0,0) tile-position matmuls execute simultaneously.
- Softmax normalizations are folded into matmuls (ones columns / ones blocks);
  ka softmax skips max-subtraction (fp32 psum); fixed shift -14 fits fp16.

Schedule: a dummy AllReduce at t=0 warms the collective path; the real
threshold AllReduce is issued right after the kv accumulation, and the qa
phase (fp8 logits + exp) + qa softmax denominators + the threshold-independent
part of the denoise/mask epilogue all execute under its latency.
"""
import os
import sys

sys.path.insert(0, "/opt/trn_rl_repo")

import numpy as np
import ml_dtypes
from contextlib import ExitStack

import concourse.bass as bass
import concourse.tile as tile
from concourse import bacc, mybir, bass_utils

f32 = mybir.dt.float32
f16 = mybir.dt.float16
f8 = mybir.dt.float8e4

B, N, DIM = 2, 8192, 512
H, A, D = 8, 256, 64
N_CORES = 8
NBLK = 4            # 2048-column blocks per batch
SHIFT = -14.0       # ka exp shift to fit fp16
WS = 64.0           # fp8 agq prescale (undone via exp scale)

_cache = {}


def _install_profile_shim():
    """Restore the axon NTFF profile hook + disable artifact upload."""
    import contextlib
    import ctypes
    import types

    if "antenv.axon_hooks" in sys.modules:
        return
    so_path = "/opt/axon/libaxon_pjrt.so"
    holder = [None]
    mod = types.ModuleType("antenv.axon_hooks")
    mod.set_axon_ntff_profile_hook = lambda h: holder.__setitem__(0, h)
    mod.get_axon_ntff_profile_hook = lambda: holder[0]
    sys.modules["antenv.axon_hooks"] = mod
    try:
        lib = ctypes.CDLL(so_path)
        if hasattr(lib, "axon_start_nrt_profile"):
            lib.axon_start_nrt_profile.argtypes = [
                ctypes.POINTER(ctypes.c_int64),
                ctypes.c_size_t,
            ]
            lib.axon_start_nrt_profile.restype = ctypes.c_int64
            lib.axon_stop_nrt_profile.argtypes = [ctypes.c_char_p]
            lib.axon_stop_nrt_profile.restype = ctypes.c_int64

            @contextlib.contextmanager
            def _hook(output_dir, device_ids):
                import jax

                jax.devices()
                if device_ids:
                    ids = (ctypes.c_int64 * len(device_ids))(*device_ids)
                    rc = lib.axon_start_nrt_profile(ids, len(device_ids))
                else:
                    rc = lib.axon_start_nrt_profile(None, 0)
                if rc != 0:
                    raise RuntimeError(f"axon_start_nrt_profile rc={rc}")
                try:
                    yield
                finally:
                    n = lib.axon_stop_nrt_profile(str(output_dir).encode())
                    if n < 0:
                        raise RuntimeError(f"axon_stop_nrt_profile rc={n}")

            mod.set_axon_ntff_profile_hook(_hook)
    except OSError:
        pass
    bass_utils.upload_artifacts = lambda tmpdir: f"file://{tmpdir}"


def _build():
    nc = bacc.Bacc("TRN2", target_bir_lowering=False, debug=False,
                   num_devices=N_CORES)

    XT = nc.dram_tensor("xt", [B * 4, 128, N], f16, kind="ExternalInput").ap()
    XT8 = nc.dram_tensor("xt8", [B * 2, 128, 2, N], f8,
                         kind="ExternalInput").ap()
    WKV = nc.dram_tensor("wkv", [4, 128, 128], f16, kind="ExternalInput").ap()
    AGQ = nc.dram_tensor("agq", [4, 128, 2, 128], f8,
                         kind="ExternalInput").ap()
    AGS = nc.dram_tensor("ags", [128, 256], f16, kind="ExternalInput").ap()
    WN = nc.dram_tensor("wn", [64, 64], f32, kind="ExternalInput").ap()
    WM = nc.dram_tensor("wm", [64, 64], f32, kind="ExternalInput").ap()
    BN = nc.dram_tensor("bn", [128, 64], f32, kind="ExternalInput").ap()
    BM = nc.dram_tensor("bm", [128, 64], f32, kind="ExternalInput").ap()
    WTT = nc.dram_tensor("wtt", [2, 128, 64], f32, kind="ExternalInput").ap()
    IDENT = nc.dram_tensor("ident", [128, 128], f32, kind="ExternalInput").ap()
    BTHR = nc.dram_tensor("bthr", [128, 1], f32, kind="ExternalInput").ap()
    VONES = nc.dram_tensor("vones", [1, N], f16, kind="ExternalInput").ap()
    OUT = nc.dram_tensor("out_t", [B, 8, 128, 512], f16,
                         kind="ExternalOutput").ap()

    EXP = mybir.ActivationFunctionType.Exp
    MULT = mybir.AluOpType.mult
    ADD = mybir.AluOpType.add
    GT = mybir.AluOpType.is_gt
    DR = mybir.MatmulPerfMode.DoubleRow
    INV_WS = 1.0 / WS

    with tile.TileContext(nc) as tc, ExitStack() as ctx:
        const = ctx.enter_context(tc.tile_pool(name="const", bufs=1))
        big = ctx.enter_context(tc.tile_pool(name="big", bufs=1))
        x8p = ctx.enter_context(tc.tile_pool(name="x8p", bufs=2))
        ek = ctx.enter_context(tc.tile_pool(name="ek", bufs=3))
        dsb = ctx.enter_context(tc.tile_pool(name="dsb", bufs=1))
        rsb = ctx.enter_context(tc.tile_pool(name="rsb", bufs=1))
        hsb = ctx.enter_context(tc.tile_pool(name="hsb", bufs=2))
        dram = ctx.enter_context(tc.tile_pool(name="dram", bufs=1, space="DRAM"))
        ps_vk = ctx.enter_context(tc.tile_pool(name="ps_vk", bufs=2, space="PSUM"))
        ps_lge = ctx.enter_context(tc.tile_pool(name="ps_lge", bufs=2, space="PSUM"))
        ps_lgo = ctx.enter_context(tc.tile_pool(name="ps_lgo", bufs=1, space="PSUM"))
        ps_kvt = ctx.enter_context(tc.tile_pool(name="ps_kvt", bufs=2, space="PSUM"))
        ps_sm = ctx.enter_context(tc.tile_pool(name="ps_sm", bufs=1, space="PSUM"))

        # ---- dummy AllReduce first: warms the collective path so the real
        # threshold AllReduce later doesn't pay one-time setup latency.
        warm_sb = dsb.tile([1, 16], f32)
        nc.vector.memset(warm_sb[:], 0.0)
        wcc_in = dram.tile([1, 16], f32)
        wcc_out = dram.tile([1, 16], f32, addr_space="Shared")
        nc.sync.dma_start(wcc_in[:], warm_sb[:])
        nc.gpsimd.collective_compute(
            "AllReduce", ADD, ins=[wcc_in[:]], outs=[wcc_out[:]],
            replica_groups=[list(range(N_CORES))])

        # ---- constants to SBUF (scalar-engine DMA queue: keeps the sync
        # queue free for the first x tiles)
        wkv_sb = []
        for dc in range(4):
            w1 = const.tile([128, 128], f16, name=f"wkv{dc}")
            nc.scalar.dma_start(w1[:], WKV[dc])
            wkv_sb.append(w1)
        agq_sb = []
        for g in range(4):
            w2 = const.tile([128, 2, 128], f8, name=f"agq{g}")
            nc.scalar.dma_start(w2[:, :, :], AGQ[g])
            agq_sb.append(w2)
        ags_sb = const.tile([128, 256], f16)
        nc.scalar.dma_start(ags_sb[:], AGS[:])
        wn_sb = const.tile([64, 64], f32)
        nc.scalar.dma_start(wn_sb[:], WN[:])
        wm_sb = const.tile([64, 64], f32)
        nc.scalar.dma_start(wm_sb[:], WM[:])
        bn_sb = const.tile([128, 64], f32)
        nc.scalar.dma_start(bn_sb[:], BN[:])
        bm_sb = const.tile([128, 64], f32)
        nc.scalar.dma_start(bm_sb[:], BM[:])
        wtt_sb = []
        for ac in range(2):
            w3 = const.tile([128, 64], f32, name=f"wtt{ac}")
            nc.scalar.dma_start(w3[:], WTT[ac])
            wtt_sb.append(w3)
        id_sb = const.tile([128, 128], f32)
        nc.scalar.dma_start(id_sb[:], IDENT[:])
        bthr_sb = const.tile([128, 1], f32)
        nc.scalar.dma_start(bthr_sb[:], BTHR[:])
        bias_sh = const.tile([128, 1], f32)
        nc.vector.memset(bias_sh[:], SHIFT)
        sones = const.tile([128, 64], f16)
        nc.vector.memset(sones[:], 1.0)
        ones128 = nc.const_aps.tensor(1.0, [128, 1])

        # ---- persistent big tiles
        # qkT rows 64:128 = k^T (all chunks); rows 0:64 cols 0:4096 = k^T of
        # odd 128-chunks (DMA partition-shifted) for the row-tile pairing.
        qkT = [big.tile([128, N], f16, name=f"qkT{b}") for b in range(B)]
        vsb = [big.tile([128, 64 * 80], f16, name=f"vsb{b}") for b in range(B)]
        vsb3 = [t[:].rearrange("p (c e) -> p c e", e=80) for t in vsb]

        # ---- x fp8 tiles for the qa phase: pre-issue loads now; the pool's
        # bufs=2 WAR pacing refills them as the qa phase consumes.
        x8_all = {}
        for blk in range(NBLK):
            for b in range(B):
                bsl = slice(blk * 2048, (blk + 1) * 2048)
                for p in range(2):
                    x8_t = x8p.tile([128, 2, 2048], f8,
                                    name=f"x8{blk}{b}{p}", tag=f"x8{b}{p}")
                    nc.gpsimd.dma_start(x8_t[:, :, :], XT8[b * 2 + p][:, :, bsl])
                    x8_all[blk, b, p] = x8_t

        # ---- phase-D/G state
        kvut = [None, None]
        zm_all = dsb.tile([128, 256], f32)    # mask logits * rs + bm
        den_all = dsb.tile([128, 256], f32)   # sigmoid denoise
        kv_all = dsb.tile([128, 256], f32)    # normalized kv
        rs_sb = {}
        r_sb = {}

        def emit_d(b, kvt_ps):
            t_u = dsb.tile([65, 256], f32, name=f"kvut{b}")
            nc.vector.tensor_copy(t_u[:], kvt_ps[b][:])
            kvut[b] = t_u
            for ac in range(2):
                g = b * 2 + ac
                gsl = slice(g * 64, (g + 1) * 64)
                asl = slice(ac * 128, (ac + 1) * 128)
                sm1 = ps_sm.tile([128, 65], f32, name="sm1", tag="sm")
                nc.tensor.matmul(sm1[:, 0:64], t_u[0:64, asl], wn_sb[:],
                                 start=True, stop=True)
                t_n = dsb.tile([128, 64], f32, name=f"noise{g}")
                nc.vector.tensor_copy(t_n[:], sm1[:, 0:64])
                sm2 = ps_sm.tile([128, 65], f32, name="sm2", tag="sm")
                nc.tensor.matmul(sm2[:, 0:64], t_u[0:64, asl], wm_sb[:],
                                 start=True, stop=True)
                t_m = dsb.tile([128, 64], f32, name=f"mask{g}")
                nc.vector.tensor_copy(t_m[:], sm2[:, 0:64])
                sm3 = ps_sm.tile([128, 65], f32, name="sm3", tag="sm")
                nc.tensor.transpose(sm3[:], t_u[:, asl], id_sb[0:65, 0:65])
                t_k = dsb.tile([128, 65], f32, name=f"kvn{g}")
                nc.vector.tensor_copy(t_k[:], sm3[:])
                t_rs = dsb.tile([128, 1], f32, name=f"rs{g}")
                nc.vector.reciprocal_approx_fast(t_rs[:], t_k[:, 64:65])
                rs_sb[b, ac] = t_rs
                nc.vector.tensor_scalar(out=kv_all[:, gsl], in0=t_k[:, 0:64],
                                        scalar1=t_rs[:], scalar2=None, op0=MULT)
                t_tmp = dsb.tile([128, 64], f32, name=f"tt{g}")
                nc.vector.tensor_tensor(t_tmp[:], kv_all[:, gsl],
                                        wtt_sb[ac][:], MULT)
                t_r = dsb.tile([128, 1], f32, name=f"r{g}")
                nc.vector.tensor_reduce(t_r[:], t_tmp[:],
                                        axis=mybir.AxisListType.X, op=ADD)
                r_sb[b, ac] = t_r
                # threshold-independent epilogue: mask pre-logit and denoise
                nc.vector.scalar_tensor_tensor(
                    out=zm_all[:, gsl], in0=t_m[:], scalar=t_rs[:],
                    in1=bm_sb[:], op0=MULT, op1=ADD)
                gn = dsb.tile([128, 64], f32, name=f"gn{g}")
                nc.vector.scalar_tensor_tensor(
                    out=gn[:], in0=t_n[:], scalar=t_rs[:],
                    in1=bn_sb[:], op0=MULT, op1=ADD)
                en = dsb.tile([128, 64], f32, name=f"en{g}")
                nc.scalar.activation(en[:], gn[:], EXP, scale=-1.0)
                dd = dsb.tile([128, 64], f32, name=f"dd{g}")
                nc.vector.tensor_scalar(out=dd[:], in0=en[:], scalar1=1.0,
                                        scalar2=None, op0=ADD)
                nc.vector.reciprocal_approx_fast(den_all[:, gsl], dd[:])

        # ====== main loop: A ([v|k] f16 proj) + B (v transpose) + C (ka) ====
        with ExitStack() as sA:
            xtp = sA.enter_context(tc.tile_pool(name="xtp", bufs=3))
            vtp = sA.enter_context(tc.tile_pool(name="vtp", bufs=3))
            kvt_ps = [ps_kvt.tile([65, 256], f32, name=f"kvtps{b}", tag="kvtps")
                      for b in range(B)]
            kv_mm_idx = [0, 0]
            # pre-issue x f16 loads; block 0 on HWDGE queues for fast start
            xts_all = {}
            for blk in range(NBLK):
                for b in range(B):
                    bsl = slice(blk * 2048, (blk + 1) * 2048)
                    for dc in range(4):
                        xt_t = xtp.tile([128, 2048], f16,
                                        name=f"x{blk}{b}{dc}", tag=f"x{dc}")
                        if blk == 0:
                            eng = nc.sync if b == 0 else nc.scalar
                        else:
                            eng = nc.gpsimd
                        eng.dma_start(xt_t[:], XT[b * 4 + dc][:, bsl])
                        xts_all[blk, b, dc] = xt_t

            for blk in range(NBLK):
                for b in range(B):
                    xts = [xts_all[blk, b, dc] for dc in range(4)]
                    vt = vtp.tile([80, 2048], f16, name="vt", tag="vt")
                    nc.sync.dma_start(vt[64:65, :], VONES[:, 0:2048])
                    # A: [v|k] projections for this 2048-col block
                    for sc in range(4):
                        nck = blk * 4 + sc
                        sl = slice(nck * 512, (nck + 1) * 512)
                        ssl = slice(sc * 512, (sc + 1) * 512)
                        vk_ps = ps_vk.tile([128, 512], f32, name="vkps",
                                           tag="vk")
                        for dc in range(4):
                            nc.tensor.matmul(vk_ps[:], wkv_sb[dc][:],
                                             xts[dc][:, ssl],
                                             start=(dc == 0), stop=(dc == 3))
                        nc.vector.tensor_copy(vt[0:64, ssl], vk_ps[0:64, :])
                        nc.vector.tensor_copy(qkT[b][64:128, sl],
                                              vk_ps[64:128, :])
                    # B: one batched xbar transpose per block
                    nc.sync.dma_start_transpose(
                        vsb3[b][:, blk * 16:(blk + 1) * 16, :], vt[:])
                    # odd chunks' k^T to partitions 0:64 (xbar partition shift)
                    ksrc = qkT[b][64:128, blk * 2048:(blk + 1) * 2048] \
                        .rearrange("p (c t e) -> p c t e", t=2, e=128)[:, :, 1, :]
                    nc.sync.dma_start(
                        qkT[b][0:64, blk * 1024:(blk + 1) * 1024], ksrc)
                    # C: paired ka logits (concurrent row tiles) -> exp -> kv^T
                    for pp in range(blk * 4, (blk + 1) * 4):
                        lg_e = ps_lge.tile([128, 512], f32, name="lge",
                                           tag="lg")
                        lg_o = ps_lgo.tile([128, 512], f32, name="lgo",
                                           tag="lg")
                        for u in range(2):
                            ce = 4 * pp + 2 * u
                            od = 2 * pp + u
                            nc.tensor.matmul(
                                lg_e[:, u * 256:(u + 1) * 256],
                                qkT[b][64:128, ce * 128:(ce + 1) * 128],
                                ags_sb[64:128, :], start=True, stop=True)
                            nc.tensor.matmul(
                                lg_o[:, u * 256:(u + 1) * 256],
                                qkT[b][0:64, od * 128:(od + 1) * 128],
                                ags_sb[0:64, :], start=True, stop=True)
                        e_e = ek.tile([128, 512], f16, name="ee", tag="eka")
                        nc.scalar.activation(e_e[:], lg_e[:], EXP,
                                             bias=bias_sh[:])
                        e_o = ek.tile([128, 512], f16, name="eo", tag="eka")
                        nc.scalar.activation(e_o[:], lg_o[:], EXP,
                                             bias=bias_sh[:])
                        for u in range(2):
                            for par, e_t in ((0, e_e), (1, e_o)):
                                c = 4 * pp + 2 * u + par
                                ki = kv_mm_idx[b]
                                nc.tensor.matmul(
                                    kvt_ps[b][:], vsb3[b][:, c, 0:65],
                                    e_t[:, u * 256:(u + 1) * 256],
                                    start=(ki == 0), stop=(ki == 63))
                                kv_mm_idx[b] += 1
                    # D: per-batch epilogue right after its last C block
                    if blk == NBLK - 1:
                        emit_d(b, kvt_ps)

            # ---- threshold partial + the real AllReduce (phases F/S below
            # execute under its latency)
            th_ps = ps_sm.tile([1, 16], f32, name="thps", tag="sm")
            k = 0
            for b in range(B):
                for ac in range(2):
                    nc.tensor.matmul(th_ps[0:1, 0:1], r_sb[b, ac][:],
                                     ones128[0:128, :],
                                     start=(k == 0), stop=(k == 3))
                    k += 1
            th_sb = dsb.tile([1, 16], f32)
            nc.vector.memset(th_sb[:], 0.0)
            nc.vector.tensor_copy(th_sb[0:1, 0:1], th_ps[0:1, 0:1])
            cc_in = dram.tile([1, 16], f32)
            cc_out = dram.tile([1, 16], f32, addr_space="Shared")
            nc.sync.dma_start(cc_in[:], th_sb[:])
            nc.gpsimd.collective_compute(
                "AllReduce", ADD, ins=[cc_in[:]], outs=[cc_out[:]],
                replica_groups=[list(range(N_CORES))])

        # ===== phase F: qa logits (fp8 DR) -> exp (under the collective) ====
        eqp = ctx.enter_context(tc.tile_pool(name="eqp", bufs=1))
        eqa = [eqp.tile([128, N], f16, name=f"eqa{b}{ac}")
               for b in range(B) for ac in range(2)]
        fq = 0
        for b in range(B):
            for blk in range(NBLK):
                for sc in range(4):
                    nck = blk * 4 + sc
                    sl = slice(nck * 512, (nck + 1) * 512)
                    ssl = slice(sc * 512, (sc + 1) * 512)
                    for ac in range(2):
                        pool = ps_lge if fq % 3 != 2 else ps_lgo
                        fq += 1
                        lgq = pool.tile([128, 512], f32, name="lgq", tag="lg")
                        for p in range(2):
                            nc.tensor.matmul(lgq[:], agq_sb[ac * 2 + p][:, :, :],
                                             x8_all[blk, b, p][:, :, ssl],
                                             start=(p == 0), stop=(p == 1),
                                             perf_mode=DR)
                        nc.scalar.activation(eqa[b * 2 + ac][:, sl],
                                             lgq[:], EXP, scale=INV_WS)

        # ===== phase S: qa softmax denominators (under the collective) ======
        rso_sb = {}
        for b in range(B):
            for pr in range(8):
                sl0 = slice((2 * pr) * 512, (2 * pr + 1) * 512)
                sl1 = slice((2 * pr + 1) * 512, (2 * pr + 2) * 512)
                s_ps = ps_vk.tile([128, 512], f32, name="sps", tag="vk")
                for ac in range(2):
                    eq = eqa[b * 2 + ac]
                    nc.tensor.matmul(s_ps[0:64, :], sones[:], eq[:, sl0],
                                     start=(ac == 0), stop=(ac == 1),
                                     tile_position=(0, 0))
                    nc.tensor.matmul(s_ps[64:128, :], sones[:], eq[:, sl1],
                                     start=(ac == 0), stop=(ac == 1),
                                     tile_position=(0, 64))
                rtmp = hsb.tile([128, 512], f32, name="rtmp", tag="rtmp")
                nc.vector.reciprocal_approx_fast(rtmp[:], s_ps[:])
                rso = rsb.tile([128, 512], f16, name=f"rso{b}{pr}")
                nc.vector.tensor_copy(rso[:], rtmp[:])
                rso_sb[b, pr] = rso

        # ---- collective result -> threshold scalar
        ts_sb = dsb.tile([1, 16], f32)
        nc.sync.dma_start(ts_sb[:], cc_out[:])
        tbc = dsb.tile([128, 1], f32)
        nc.gpsimd.partition_broadcast(tbc[:], ts_sb[0:1, 0:1])
        tfin = dsb.tile([128, 1], f32)
        nc.vector.tensor_scalar(out=tfin[:], in0=tbc[:],
                                scalar1=1.0 / (B * A), scalar2=bthr_sb[:],
                                op0=MULT, op1=ADD)

        # ========== phase G: thresholded mask + second softmax ==============
        mb = dsb.tile([128, 256], f32)
        nc.vector.tensor_scalar(out=mb[:], in0=zm_all[:], scalar1=tfin[:],
                                scalar2=None, op0=GT)
        kvm = dsb.tile([128, 256], f32)
        nc.vector.tensor_tensor(kvm[:], kv_all[:], mb[:], MULT)
        l2 = dsb.tile([128, 256], f32)
        nc.vector.tensor_tensor(l2[:], kvm[:], den_all[:], ADD)
        e2 = dsb.tile([128, 256], f32)
        s24 = dsb.tile([128, 4], f32)
        for g in range(4):
            gsl = slice(g * 64, (g + 1) * 64)
            nc.scalar.activation(e2[:, gsl], l2[:, gsl], EXP,
                                 accum_out=s24[:, g:g + 1])
        rs24 = dsb.tile([128, 4], f32)
        nc.vector.reciprocal_approx_fast(rs24[:], s24[:])
        kv2 = dsb.tile([128, 256], f16)
        for g in range(4):
            gsl = slice(g * 64, (g + 1) * 64)
            nc.vector.tensor_scalar(out=kv2[:, gsl], in0=e2[:, gsl],
                                    scalar1=rs24[:, g:g + 1], scalar2=None,
                                    op0=MULT)

        # ===== phase H: out^T = kv2^T @ E_qa^T, paired via column tiling ====
        for b in range(B):
            for pr in range(8):
                sl0 = slice((2 * pr) * 512, (2 * pr + 1) * 512)
                sl1 = slice((2 * pr + 1) * 512, (2 * pr + 2) * 512)
                pool = ps_lge if pr % 2 == 0 else ps_vk
                tg = "lg" if pr % 2 == 0 else "vk"
                o_ps = pool.tile([128, 512], f32, name="ops", tag=tg)
                for ac in range(2):
                    k2 = kv2[:, (b * 2 + ac) * 64:(b * 2 + ac + 1) * 64]
                    eq = eqa[b * 2 + ac]
                    nc.tensor.matmul(o_ps[0:64, :], k2, eq[:, sl0],
                                     start=(ac == 0), stop=(ac == 1),
                                     tile_position=(0, 0))
                    nc.tensor.matmul(o_ps[64:128, :], k2, eq[:, sl1],
                                     start=(ac == 0), stop=(ac == 1),
                                     tile_position=(0, 64))
                ob = hsb.tile([128, 512], f16, name="ob", tag="ob")
                nc.vector.tensor_tensor(ob[:], o_ps[:], rso_sb[b, pr][:], MULT)
                nc.scalar.dma_start(OUT[b, pr], ob[:])

    nc.compile()
    return nc


def _prep_inputs(x, w_qkv, agent, w_noise, b_noise, w_mask, b_mask,
                 w_thresh, b_thresh):
    scale = D ** -0.5
    xt = np.ascontiguousarray(
        x.transpose(0, 2, 1).astype(np.float16)).reshape(B * 4, 128, N)
    # fp8 x, dim-chunks paired for DoubleRow: [b, pair, row, plane, n]
    xt8 = np.ascontiguousarray(
        x.transpose(0, 2, 1).reshape(B, 2, 2, 128, N).transpose(0, 1, 3, 2, 4)
        .reshape(B * 2, 128, 2, N)).astype(ml_dtypes.float8_e4m3)
    wq = w_qkv[0:H * D].reshape(H, D, DIM)
    wk = w_qkv[H * D:2 * H * D].reshape(H, D, DIM)
    wv = w_qkv[2 * H * D:3 * H * D].reshape(H, D, DIM)
    bn_rep = np.ascontiguousarray(
        np.broadcast_to(b_noise[None, :], (128, 64))).astype(np.float32)
    bm_rep = np.ascontiguousarray(
        np.broadcast_to(b_mask[None, :], (128, 64))).astype(np.float32)
    wtt = np.zeros((A, D), np.float32)
    for a in range(A):
        wtt[a] = w_thresh[0, (a % 8) * D:(a % 8 + 1) * D]
    wtt = wtt.reshape(2, 128, 64)
    ident = np.eye(128, dtype=np.float32)
    vones = np.ones((1, N), np.float16)
    bthr = np.full((128, 1), float(np.asarray(b_thresh).ravel()[0]), np.float32)
    in_maps = []
    for h in range(H):
        wvk_h = np.concatenate([wv[h], wk[h]], axis=0)            # [128, 512]
        wvk_t = np.ascontiguousarray(wvk_h.T).astype(np.float16)  # [512, 128]
        # folded fp8 q-agent: [DIM, A], prescaled by WS
        agq = np.einsum('dc,ad->ca', wq[h].astype(np.float64),
                        agent[h].astype(np.float64)) * (scale * WS)
        # build AGQ [g = ac*2 + pair, row, plane, col]
        agq4 = np.empty((4, 128, 2, 128), np.float32)
        for ac in range(2):
            for p in range(2):
                for pl in range(2):
                    rows = slice(p * 256 + pl * 128, p * 256 + (pl + 1) * 128)
                    agq4[ac * 2 + p, :, pl, :] = \
                        agq[rows, ac * 128:(ac + 1) * 128]
        agq4 = agq4.astype(ml_dtypes.float8_e4m3)
        ags = np.ascontiguousarray(np.concatenate(
            [agent[h].T, agent[h].T], axis=0)).astype(np.float16)
        in_maps.append({
            "xt": xt,
            "xt8": xt8,
            "wkv": np.ascontiguousarray(wvk_t.reshape(4, 128, 128)),
            "agq": agq4,
            "ags": ags,
            "wn": np.ascontiguousarray(w_noise.T).astype(np.float32),
            "wm": np.ascontiguousarray(w_mask.T).astype(np.float32),
            "bn": bn_rep,
            "bm": bm_rep,
            "wtt": wtt,
            "ident": ident,
            "bthr": bthr,
            "vones": vones,
        })
    return in_maps


LAST_EXEC_NS = None
LAST_RES = None


def kernel(**inputs):
    global LAST_EXEC_NS, LAST_RES
    _install_profile_shim()
    if "nc" not in _cache:
        _cache["nc"] = _build()
    nc = _cache["nc"]
    inputs = {k: np.asarray(v) for k, v in inputs.items()}
    in_maps = _prep_inputs(**inputs)
    trace = os.environ.get("BASS_KERNEL_TRACE", "0") == "1"
    res = bass_utils.run_bass_kernel_spmd(
        nc, in_maps, core_ids=list(range(N_CORES)), trace=trace)
    LAST_EXEC_NS = res.exec_time_ns
    LAST_RES = res
    out = np.empty((B, N, H * D), np.float32)
    for h in range(H):
        o = np.asarray(res.results[h]["out_t"]).astype(np.float32)
        # row p<64 of pair pr -> (n = 1024*pr + c, d = p);
        # row p>=64        -> (n = 1024*pr + 512 + c, d = p - 64)
        o2 = o.reshape(B, 8, 2, 64, 512)     # [b, pr, half, d, c]
        o3 = o2.transpose(0, 1, 2, 4, 3).reshape(B, N, D)
        out[:, :, h * D:(h + 1) * D] = o3
    return out


# revision 10
# speedup vs baseline: 1.1919x; 1.1919x over previous
"""Trainium2 Bass kernel for nn_Agent_Aggregator_with_Mask_Denoise_Mechanism.

Sharding: tensor-parallel over heads h (8 heads -> 8 cores). Each core computes
its head end-to-end; the only cross-core value is the scalar threshold logit
(an 8-way AllReduce of a 4-byte partial sum). Host does the final (h d)
concat + transpose.

Math notes vs the reference:
- sigmoid(m) > sigmoid(t)  <=>  m > t: the mask threshold compares pre-sigmoid
  logits, so no sigmoid tables are needed (Exp only).
- The reference's kv.reshape(b, a, h*d) row-major remap means the thresh
  weight applied to kv[b,h,a,d] is w_thresh[0, (a%8)*64+d], independent of h.
- q is never materialized: qa logits = x @ (wq^T agent^T scale), the folded
  matrix precomputed on host, prescaled by 64, fp8 DoubleRow (2x rate);
  exp(logit/64) undoes the scale. k/v stay f16 (fp8 breaks the 2e-2 gate)
  but share one 128-wide stationary [v|k] so the PE array is fully used.
- Softmax normalizations fold into matmuls (ones columns / ones blocks);
  ka softmax skips max-subtraction (fp32 psum); fixed shift -14 fits fp16.

Perf notes (this device is DMA-packet-slot bound at ~87 packets/us, 4KB max
per packet, in addition to the PE):
- v^T -> v transposes run on the PE (transpose mode) instead of the DMA xbar:
  the xbar emits 256-B packets (1280 slots per block = most of the budget).
  The ones column of the kv stationary comes from memset-ing vsb to 1.0 once.
- x is loaded once in f16 (4KB packets) + once in fp8 pairs for the qa phase.
- The output is staged into one [128, 4096] SBUF tile per batch and shipped
  with a single large DMA.
- A dummy AllReduce at t=0 warms the collective path; the real AllReduce is
  issued right after the kv accumulation with the qa-denominator phase and
  the threshold-independent denoise epilogue executing under its latency.
- The threshold broadcast to 128 partitions is a [1,128]-ones matmul, not
  gpsimd partition_broadcast (measured ~15us there).
"""
import os
import sys

sys.path.insert(0, "/opt/trn_rl_repo")

import numpy as np
import ml_dtypes
from contextlib import ExitStack

import concourse.bass as bass
import concourse.tile as tile
from concourse import bacc, mybir, bass_utils

f32 = mybir.dt.float32
f16 = mybir.dt.float16
f8 = mybir.dt.float8e4

B, N, DIM = 2, 8192, 512
H, A, D = 8, 256, 64
N_CORES = 8
NBLK = 4            # 2048-column blocks per batch
SHIFT = -14.0       # ka exp shift to fit fp16
WS = 64.0           # fp8 agq prescale (undone via exp scale)

_cache = {}


def _install_profile_shim():
    """Restore the axon NTFF profile hook + disable artifact upload."""
    import contextlib
    import ctypes
    import types

    if "antenv.axon_hooks" in sys.modules:
        return
    so_path = "/opt/axon/libaxon_pjrt.so"
    holder = [None]
    mod = types.ModuleType("antenv.axon_hooks")
    mod.set_axon_ntff_profile_hook = lambda h: holder.__setitem__(0, h)
    mod.get_axon_ntff_profile_hook = lambda: holder[0]
    sys.modules["antenv.axon_hooks"] = mod
    try:
        lib = ctypes.CDLL(so_path)
        if hasattr(lib, "axon_start_nrt_profile"):
            lib.axon_start_nrt_profile.argtypes = [
                ctypes.POINTER(ctypes.c_int64),
                ctypes.c_size_t,
            ]
            lib.axon_start_nrt_profile.restype = ctypes.c_int64
            lib.axon_stop_nrt_profile.argtypes = [ctypes.c_char_p]
            lib.axon_stop_nrt_profile.restype = ctypes.c_int64

            @contextlib.contextmanager
            def _hook(output_dir, device_ids):
                import jax

                jax.devices()
                if device_ids:
                    ids = (ctypes.c_int64 * len(device_ids))(*device_ids)
                    rc = lib.axon_start_nrt_profile(ids, len(device_ids))
                else:
                    rc = lib.axon_start_nrt_profile(None, 0)
                if rc != 0:
                    raise RuntimeError(f"axon_start_nrt_profile rc={rc}")
                try:
                    yield
                finally:
                    n = lib.axon_stop_nrt_profile(str(output_dir).encode())
                    if n < 0:
                        raise RuntimeError(f"axon_stop_nrt_profile rc={n}")

            mod.set_axon_ntff_profile_hook(_hook)
    except OSError:
        pass
    bass_utils.upload_artifacts = lambda tmpdir: f"file://{tmpdir}"


def _build():
    nc = bacc.Bacc("TRN2", target_bir_lowering=False, debug=False,
                   num_devices=N_CORES)

    XT = nc.dram_tensor("xt", [B * 4, 128, N], f16, kind="ExternalInput").ap()
    XT8 = nc.dram_tensor("xt8", [B * 2, 128, 2, N], f8,
                         kind="ExternalInput").ap()
    WKV = nc.dram_tensor("wkv", [4, 128, 128], f16, kind="ExternalInput").ap()
    AGQ = nc.dram_tensor("agq", [4, 128, 2, 128], f8,
                         kind="ExternalInput").ap()
    AGS = nc.dram_tensor("ags", [128, 256], f16, kind="ExternalInput").ap()
    WN = nc.dram_tensor("wn", [64, 64], f32, kind="ExternalInput").ap()
    WM = nc.dram_tensor("wm", [64, 64], f32, kind="ExternalInput").ap()
    BN = nc.dram_tensor("bn", [128, 64], f32, kind="ExternalInput").ap()
    BM = nc.dram_tensor("bm", [128, 64], f32, kind="ExternalInput").ap()
    WTT = nc.dram_tensor("wtt", [2, 128, 64], f32, kind="ExternalInput").ap()
    IDENT = nc.dram_tensor("ident", [128, 128], f32, kind="ExternalInput").ap()
    ID16 = nc.dram_tensor("id16", [64, 64], f16, kind="ExternalInput").ap()
    BTHR = nc.dram_tensor("bthr", [128, 1], f32, kind="ExternalInput").ap()
    OUT = nc.dram_tensor("out_t", [B, 128, 4096], f16,
                         kind="ExternalOutput").ap()

    EXP = mybir.ActivationFunctionType.Exp
    MULT = mybir.AluOpType.mult
    ADD = mybir.AluOpType.add
    GT = mybir.AluOpType.is_gt
    DR = mybir.MatmulPerfMode.DoubleRow
    INV_WS = 1.0 / WS

    with tile.TileContext(nc) as tc, ExitStack() as ctx:
        const = ctx.enter_context(tc.tile_pool(name="const", bufs=1))
        big = ctx.enter_context(tc.tile_pool(name="big", bufs=1))
        x8p = ctx.enter_context(tc.tile_pool(name="x8p", bufs=2))
        ek = ctx.enter_context(tc.tile_pool(name="ek", bufs=2))
        dsb = ctx.enter_context(tc.tile_pool(name="dsb", bufs=1))
        dram = ctx.enter_context(tc.tile_pool(name="dram", bufs=1, space="DRAM"))
        ps_vk = ctx.enter_context(tc.tile_pool(name="ps_vk", bufs=2, space="PSUM"))
        ps_lg = ctx.enter_context(tc.tile_pool(name="ps_lg", bufs=3, space="PSUM"))
        ps_kvt = ctx.enter_context(tc.tile_pool(name="ps_kvt", bufs=2, space="PSUM"))
        ps_sm = ctx.enter_context(tc.tile_pool(name="ps_sm", bufs=1, space="PSUM"))

        # ---- dummy AllReduce first: warms the collective path
        warm_sb = dsb.tile([1, 16], f32)
        nc.vector.memset(warm_sb[:], 0.0)
        wcc_in = dram.tile([1, 16], f32)
        wcc_out = dram.tile([1, 16], f32, addr_space="Shared")
        nc.sync.dma_start(wcc_in[:], warm_sb[:])
        nc.gpsimd.collective_compute(
            "AllReduce", ADD, ins=[wcc_in[:]], outs=[wcc_out[:]],
            replica_groups=[list(range(N_CORES))])

        # ---- constants to SBUF (scalar-engine DMA queue)
        wkv_sb = []
        for dc in range(4):
            w1 = const.tile([128, 128], f16, name=f"wkv{dc}")
            nc.scalar.dma_start(w1[:], WKV[dc])
            wkv_sb.append(w1)
        agq_sb = []
        for g in range(4):
            w2 = const.tile([128, 2, 128], f8, name=f"agq{g}")
            nc.scalar.dma_start(w2[:, :, :], AGQ[g])
            agq_sb.append(w2)
        ags_sb = const.tile([128, 256], f16)
        nc.scalar.dma_start(ags_sb[:], AGS[:])
        wn_sb = const.tile([64, 64], f32)
        nc.scalar.dma_start(wn_sb[:], WN[:])
        wm_sb = const.tile([64, 64], f32)
        nc.scalar.dma_start(wm_sb[:], WM[:])
        bn_sb = const.tile([128, 64], f32)
        nc.scalar.dma_start(bn_sb[:], BN[:])
        bm_sb = const.tile([128, 64], f32)
        nc.scalar.dma_start(bm_sb[:], BM[:])
        wtt_sb = []
        for ac in range(2):
            w3 = const.tile([128, 64], f32, name=f"wtt{ac}")
            nc.scalar.dma_start(w3[:], WTT[ac])
            wtt_sb.append(w3)
        id_sb = const.tile([128, 128], f32)
        nc.scalar.dma_start(id_sb[:], IDENT[:])
        id16_sb = const.tile([64, 64], f16)
        nc.scalar.dma_start(id16_sb[:], ID16[:])
        bthr_sb = const.tile([128, 1], f32)
        nc.scalar.dma_start(bthr_sb[:], BTHR[:])
        bias_sh = const.tile([128, 1], f32)
        nc.vector.memset(bias_sh[:], SHIFT)
        sones = const.tile([128, 64], f16)
        nc.vector.memset(sones[:], 1.0)
        ones128 = nc.const_aps.tensor(1.0, [128, 1])
        ones_bc = nc.const_aps.tensor(1.0, [1, 128])

        # ---- persistent big tiles
        qkT = [big.tile([128, N], f16, name=f"qkT{b}") for b in range(B)]
        vsb = [big.tile([128, 64 * 65], f16, name=f"vsb{b}") for b in range(B)]
        vsb3 = [t[:].rearrange("p (c e) -> p c e", e=65) for t in vsb]
        for b in range(B):
            nc.vector.memset(vsb[b][:], 1.0)    # ones column for kv stat
        eqa = [big.tile([128, N], f16, name=f"eqa{b}{ac}")
               for b in range(B) for ac in range(2)]

        # ---- x fp8 pair tiles for the qa phase (pre-issued, WAR-paced)
        x8_all = {}
        for blk in range(NBLK):
            for b in range(B):
                bsl = slice(blk * 2048, (blk + 1) * 2048)
                for p in range(2):
                    x8_t = x8p.tile([128, 2, 2048], f8,
                                    name=f"x8{blk}{b}{p}", tag=f"x8{b}{p}")
                    nc.gpsimd.dma_start(x8_t[:, :, :], XT8[b * 2 + p][:, :, bsl])
                    x8_all[blk, b, p] = x8_t

        # ---- phase-D/G state
        kvut = [None, None]
        zm_all = dsb.tile([128, 256], f32)    # mask logits * rs + bm
        den_all = dsb.tile([128, 256], f32)   # sigmoid denoise
        kv_all = dsb.tile([128, 256], f32)    # normalized kv
        rs_sb = {}
        r_sb = {}

        def emit_d(b, kvt_ps):
            t_u = dsb.tile([65, 256], f32, name=f"kvut{b}")
            nc.vector.tensor_copy(t_u[:], kvt_ps[b][:])
            kvut[b] = t_u
            for ac in range(2):
                g = b * 2 + ac
                gsl = slice(g * 64, (g + 1) * 64)
                asl = slice(ac * 128, (ac + 1) * 128)
                sm1 = ps_sm.tile([128, 65], f32, name="sm1", tag="sm")
                nc.tensor.matmul(sm1[:, 0:64], t_u[0:64, asl], wn_sb[:],
                                 start=True, stop=True)
                t_n = dsb.tile([128, 64], f32, name=f"noise{g}")
                nc.vector.tensor_copy(t_n[:], sm1[:, 0:64])
                sm2 = ps_sm.tile([128, 65], f32, name="sm2", tag="sm")
                nc.tensor.matmul(sm2[:, 0:64], t_u[0:64, asl], wm_sb[:],
                                 start=True, stop=True)
                t_m = dsb.tile([128, 64], f32, name=f"mask{g}")
                nc.vector.tensor_copy(t_m[:], sm2[:, 0:64])
                sm3 = ps_sm.tile([128, 65], f32, name="sm3", tag="sm")
                nc.tensor.transpose(sm3[:], t_u[:, asl], id_sb[0:65, 0:65])
                t_k = dsb.tile([128, 65], f32, name=f"kvn{g}")
                nc.vector.tensor_copy(t_k[:], sm3[:])
                t_rs = dsb.tile([128, 1], f32, name=f"rs{g}")
                nc.vector.reciprocal_approx_fast(t_rs[:], t_k[:, 64:65])
                rs_sb[b, ac] = t_rs
                nc.vector.tensor_scalar(out=kv_all[:, gsl], in0=t_k[:, 0:64],
                                        scalar1=t_rs[:], scalar2=None, op0=MULT)
                t_tmp = dsb.tile([128, 64], f32, name=f"tt{g}")
                nc.vector.tensor_tensor(t_tmp[:], kv_all[:, gsl],
                                        wtt_sb[ac][:], MULT)
                t_r = dsb.tile([128, 1], f32, name=f"r{g}")
                nc.vector.tensor_reduce(t_r[:], t_tmp[:],
                                        axis=mybir.AxisListType.X, op=ADD)
                r_sb[b, ac] = t_r
                # threshold-independent epilogue: mask pre-logit and denoise
                nc.vector.scalar_tensor_tensor(
                    out=zm_all[:, gsl], in0=t_m[:], scalar=t_rs[:],
                    in1=bm_sb[:], op0=MULT, op1=ADD)
                gn = dsb.tile([128, 64], f32, name=f"gn{g}")
                nc.vector.scalar_tensor_tensor(
                    out=gn[:], in0=t_n[:], scalar=t_rs[:],
                    in1=bn_sb[:], op0=MULT, op1=ADD)
                en = dsb.tile([128, 64], f32, name=f"en{g}")
                nc.scalar.activation(en[:], gn[:], EXP, scale=-1.0)
                dd = dsb.tile([128, 64], f32, name=f"dd{g}")
                nc.vector.tensor_scalar(out=dd[:], in0=en[:], scalar1=1.0,
                                        scalar2=None, op0=ADD)
                nc.vector.reciprocal_approx_fast(den_all[:, gsl], dd[:])

        # == main loop: A ([v|k] proj) + T (PE transpose) + C (ka/kv) + F (qa)
        with ExitStack() as sA:
            xtp = sA.enter_context(tc.tile_pool(name="xtp", bufs=2))
            vtp = sA.enter_context(tc.tile_pool(name="vtp", bufs=1))
            kvt_ps = [ps_kvt.tile([65, 256], f32, name=f"kvtps{b}", tag="kvtps")
                      for b in range(B)]
            kv_mm_idx = [0, 0]
            # pre-issue x f16 loads; b0 on sync HWDGE, b1 on scalar HWDGE
            xts_all = {}
            for blk in range(NBLK):
                for b in range(B):
                    bsl = slice(blk * 2048, (blk + 1) * 2048)
                    for dc in range(4):
                        xt_t = xtp.tile([128, 2048], f16,
                                        name=f"x{blk}{b}{dc}", tag=f"x{dc}")
                        eng = nc.sync if b == 0 else nc.scalar
                        eng.dma_start(xt_t[:], XT[b * 4 + dc][:, bsl])
                        xts_all[blk, b, dc] = xt_t

            for blk in range(NBLK):
                for b in range(B):
                    xts = [xts_all[blk, b, dc] for dc in range(4)]
                    vt = vtp.tile([64, 2048], f16, name="vt", tag="vt")
                    # A: [v|k] projections for this 2048-col block
                    for sc in range(4):
                        nck = blk * 4 + sc
                        sl = slice(nck * 512, (nck + 1) * 512)
                        ssl = slice(sc * 512, (sc + 1) * 512)
                        vk_ps = ps_vk.tile([128, 512], f32, name="vkps",
                                           tag="vk")
                        for dc in range(4):
                            nc.tensor.matmul(vk_ps[:], wkv_sb[dc][:],
                                             xts[dc][:, ssl],
                                             start=(dc == 0), stop=(dc == 3))
                        nc.vector.tensor_copy(vt[:, ssl], vk_ps[0:64, :])
                        nc.vector.tensor_copy(qkT[b][64:128, sl],
                                              vk_ps[64:128, :])
                    # T: PE-mode transposes, 4 chunks per psum tile
                    for tg in range(4):
                        tr_ps = ps_vk.tile([128, 4, 64], f16, name="trps",
                                           tag="vk")
                        for j in range(4):
                            cc = tg * 4 + j
                            nc.tensor.transpose(
                                tr_ps[:, j, :], vt[:, cc * 128:(cc + 1) * 128],
                                id16_sb[:])
                        c0 = blk * 16 + tg * 4
                        nc.vector.tensor_copy(
                            vsb3[b][:, c0:c0 + 4, 0:64], tr_ps[:, :, :])
                    # C: ka logits -> exp -> kv^T accumulation
                    for cp in range(blk * 8, (blk + 1) * 8):
                        lg = ps_lg.tile([128, 512], f32, name="lg", tag="lg")
                        for j in range(2):
                            c = 2 * cp + j
                            nc.tensor.matmul(
                                lg[:, j * 256:(j + 1) * 256],
                                qkT[b][64:128, c * 128:(c + 1) * 128],
                                ags_sb[64:128, :],
                                start=True, stop=True)
                        e_t = ek.tile([128, 512], f16, name="eka", tag="eka")
                        nc.scalar.activation(e_t[:], lg[:], EXP,
                                             bias=bias_sh[:])
                        for j in range(2):
                            c = 2 * cp + j
                            ki = kv_mm_idx[b]
                            nc.tensor.matmul(
                                kvt_ps[b][:], vsb3[b][:, c, 0:65],
                                e_t[:, j * 256:(j + 1) * 256],
                                start=(ki == 0), stop=(ki == 63))
                            kv_mm_idx[b] += 1
                    # F: qa logits (fp8 DoubleRow) -> exp for this block
                    for sc in range(4):
                        nck = blk * 4 + sc
                        sl = slice(nck * 512, (nck + 1) * 512)
                        ssl = slice(sc * 512, (sc + 1) * 512)
                        for ac in range(2):
                            lgq = ps_lg.tile([128, 512], f32, name="lgq",
                                             tag="lg")
                            for p in range(2):
                                nc.tensor.matmul(
                                    lgq[:], agq_sb[ac * 2 + p][:, :, :],
                                    x8_all[blk, b, p][:, :, ssl],
                                    start=(p == 0), stop=(p == 1),
                                    perf_mode=DR)
                            nc.scalar.activation(eqa[b * 2 + ac][:, sl],
                                                 lgq[:], EXP, scale=INV_WS)
                    # D: per-batch epilogue right after its last C block
                    if blk == NBLK - 1:
                        emit_d(b, kvt_ps)

            # ---- threshold partial + the real AllReduce (phase S below
            # executes under its latency)
            th_ps = ps_sm.tile([1, 16], f32, name="thps", tag="sm")
            k = 0
            for b in range(B):
                for ac in range(2):
                    nc.tensor.matmul(th_ps[0:1, 0:1], r_sb[b, ac][:],
                                     ones128[0:128, :],
                                     start=(k == 0), stop=(k == 3))
                    k += 1
            th_sb = dsb.tile([1, 16], f32)
            nc.vector.memset(th_sb[:], 0.0)
            nc.vector.tensor_copy(th_sb[0:1, 0:1], th_ps[0:1, 0:1])
            cc_in = dram.tile([1, 16], f32)
            cc_out = dram.tile([1, 16], f32, addr_space="Shared")
            nc.sync.dma_start(cc_in[:], th_sb[:])
            nc.gpsimd.collective_compute(
                "AllReduce", ADD, ins=[cc_in[:]], outs=[cc_out[:]],
                replica_groups=[list(range(N_CORES))])

        # ===== phase S: qa softmax denominators (under the collective) ======
        rsb = ctx.enter_context(tc.tile_pool(name="rsb", bufs=1))
        hsb = ctx.enter_context(tc.tile_pool(name="hsb", bufs=2))
        rso_sb = {}
        for b in range(B):
            for pr in range(8):
                sl0 = slice((2 * pr) * 512, (2 * pr + 1) * 512)
                sl1 = slice((2 * pr + 1) * 512, (2 * pr + 2) * 512)
                s_ps = ps_vk.tile([128, 512], f32, name="sps", tag="vk")
                for ac in range(2):
                    eq = eqa[b * 2 + ac]
                    nc.tensor.matmul(s_ps[0:64, :], sones[:], eq[:, sl0],
                                     start=(ac == 0), stop=(ac == 1),
                                     tile_position=(0, 0))
                    nc.tensor.matmul(s_ps[64:128, :], sones[:], eq[:, sl1],
                                     start=(ac == 0), stop=(ac == 1),
                                     tile_position=(0, 64))
                rtmp = hsb.tile([128, 512], f32, name="rtmp", tag="rtmp")
                nc.vector.reciprocal_approx_fast(rtmp[:], s_ps[:])
                rso = rsb.tile([128, 512], f16, name=f"rso{b}{pr}")
                nc.vector.tensor_copy(rso[:], rtmp[:])
                rso_sb[b, pr] = rso

        # ---- collective result -> threshold scalar (PE broadcast)
        ts_sb = dsb.tile([1, 16], f32)
        nc.sync.dma_start(ts_sb[:], cc_out[:])
        tb_ps = ps_sm.tile([128, 16], f32, name="tbps", tag="sm")
        nc.tensor.matmul(tb_ps[:, 0:1], ones_bc[:], ts_sb[0:1, 0:1],
                         start=True, stop=True)
        tfin = dsb.tile([128, 1], f32)
        nc.vector.tensor_scalar(out=tfin[:], in0=tb_ps[:, 0:1],
                                scalar1=1.0 / (B * A), scalar2=bthr_sb[:],
                                op0=MULT, op1=ADD)

        # ========== phase G: thresholded mask + second softmax ==============
        mb = dsb.tile([128, 256], f32)
        nc.vector.tensor_scalar(out=mb[:], in0=zm_all[:], scalar1=tfin[:],
                                scalar2=None, op0=GT)
        kvm = dsb.tile([128, 256], f32)
        nc.vector.tensor_tensor(kvm[:], kv_all[:], mb[:], MULT)
        l2 = dsb.tile([128, 256], f32)
        nc.vector.tensor_tensor(l2[:], kvm[:], den_all[:], ADD)
        e2 = dsb.tile([128, 256], f32)
        s24 = dsb.tile([128, 4], f32)
        for g in range(4):
            gsl = slice(g * 64, (g + 1) * 64)
            nc.scalar.activation(e2[:, gsl], l2[:, gsl], EXP,
                                 accum_out=s24[:, g:g + 1])
        rs24 = dsb.tile([128, 4], f32)
        nc.vector.reciprocal_approx_fast(rs24[:], s24[:])
        kv2 = dsb.tile([128, 256], f16)
        for g in range(4):
            gsl = slice(g * 64, (g + 1) * 64)
            nc.vector.tensor_scalar(out=kv2[:, gsl], in0=e2[:, gsl],
                                    scalar1=rs24[:, g:g + 1], scalar2=None,
                                    op0=MULT)

        # ===== phase H: out^T = kv2^T @ E_qa^T, staged output ===============
        ostg = [rsb.tile([128, 4096], f16, name=f"ostg{b}") for b in range(B)]
        for b in range(B):
            for pr in range(8):
                sl0 = slice((2 * pr) * 512, (2 * pr + 1) * 512)
                sl1 = slice((2 * pr + 1) * 512, (2 * pr + 2) * 512)
                pool = ps_lg if pr % 2 == 0 else ps_vk
                tg2 = "lg" if pr % 2 == 0 else "vk"
                o_ps = pool.tile([128, 512], f32, name="ops", tag=tg2)
                for ac in range(2):
                    k2 = kv2[:, (b * 2 + ac) * 64:(b * 2 + ac + 1) * 64]
                    eq = eqa[b * 2 + ac]
                    nc.tensor.matmul(o_ps[0:64, :], k2, eq[:, sl0],
                                     start=(ac == 0), stop=(ac == 1),
                                     tile_position=(0, 0))
                    nc.tensor.matmul(o_ps[64:128, :], k2, eq[:, sl1],
                                     start=(ac == 0), stop=(ac == 1),
                                     tile_position=(0, 64))
                nc.vector.tensor_tensor(ostg[b][:, pr * 512:(pr + 1) * 512],
                                        o_ps[:], rso_sb[b, pr][:], MULT)
            nc.scalar.dma_start(OUT[b], ostg[b][:])

    nc.compile()
    return nc


def _prep_inputs(x, w_qkv, agent, w_noise, b_noise, w_mask, b_mask,
                 w_thresh, b_thresh):
    scale = D ** -0.5
    xt = np.ascontiguousarray(
        x.transpose(0, 2, 1).astype(np.float16)).reshape(B * 4, 128, N)
    xt8 = np.ascontiguousarray(
        x.transpose(0, 2, 1).reshape(B, 2, 2, 128, N).transpose(0, 1, 3, 2, 4)
        .reshape(B * 2, 128, 2, N)).astype(ml_dtypes.float8_e4m3)
    wq = w_qkv[0:H * D].reshape(H, D, DIM)
    wk = w_qkv[H * D:2 * H * D].reshape(H, D, DIM)
    wv = w_qkv[2 * H * D:3 * H * D].reshape(H, D, DIM)
    bn_rep = np.ascontiguousarray(
        np.broadcast_to(b_noise[None, :], (128, 64))).astype(np.float32)
    bm_rep = np.ascontiguousarray(
        np.broadcast_to(b_mask[None, :], (128, 64))).astype(np.float32)
    wtt = np.zeros((A, D), np.float32)
    for a in range(A):
        wtt[a] = w_thresh[0, (a % 8) * D:(a % 8 + 1) * D]
    wtt = wtt.reshape(2, 128, 64)
    ident = np.eye(128, dtype=np.float32)
    id16 = np.eye(64, dtype=np.float16)
    bthr = np.full((128, 1), float(np.asarray(b_thresh).ravel()[0]), np.float32)
    in_maps = []
    for h in range(H):
        wvk_h = np.concatenate([wv[h], wk[h]], axis=0)            # [128, 512]
        wvk_t = np.ascontiguousarray(wvk_h.T).astype(np.float16)  # [512, 128]
        agq = np.einsum('dc,ad->ca', wq[h].astype(np.float64),
                        agent[h].astype(np.float64)) * (scale * WS)
        agq = agq.astype(np.float32)                              # [DIM, A]
        agq4 = np.empty((4, 128, 2, 128), np.float32)
        for ac in range(2):
            for p in range(2):
                for pl in range(2):
                    rows = slice(p * 256 + pl * 128, p * 256 + (pl + 1) * 128)
                    agq4[ac * 2 + p, :, pl, :] = \
                        agq[rows, ac * 128:(ac + 1) * 128]
        agq4 = agq4.astype(ml_dtypes.float8_e4m3)
        ags = np.ascontiguousarray(np.concatenate(
            [agent[h].T, agent[h].T], axis=0)).astype(np.float16)
        in_maps.append({
            "xt": xt,
            "xt8": xt8,
            "wkv": np.ascontiguousarray(wvk_t.reshape(4, 128, 128)),
            "agq": agq4,
            "ags": ags,
            "wn": np.ascontiguousarray(w_noise.T).astype(np.float32),
            "wm": np.ascontiguousarray(w_mask.T).astype(np.float32),
            "bn": bn_rep,
            "bm": bm_rep,
            "wtt": wtt,
            "ident": ident,
            "id16": id16,
            "bthr": bthr,
        })
    return in_maps


LAST_EXEC_NS = None
LAST_RES = None


def kernel(**inputs):
    global LAST_EXEC_NS, LAST_RES
    _install_profile_shim()
    if "nc" not in _cache:
        _cache["nc"] = _build()
    nc = _cache["nc"]
    inputs = {k: np.asarray(v) for k, v in inputs.items()}
    in_maps = _prep_inputs(**inputs)
    trace = os.environ.get("BASS_KERNEL_TRACE", "0") == "1"
    res = bass_utils.run_bass_kernel_spmd(
        nc, in_maps, core_ids=list(range(N_CORES)), trace=trace)
    LAST_EXEC_NS = res.exec_time_ns
    LAST_RES = res
    out = np.empty((B, N, H * D), np.float32)
    for h in range(H):
        o = np.asarray(res.results[h]["out_t"]).astype(np.float32)
        # o[b, (half,d), (pr,c)]: n = pr*1024 + half*512 + c
        o2 = o.reshape(B, 2, 64, 8, 512)         # [b, half, d, pr, c]
        o3 = o2.transpose(0, 3, 1, 4, 2).reshape(B, N, D)
        out[:, :, h * D:(h + 1) * D] = o3
    return out


# revision 12
# speedup vs baseline: 1.4920x; 1.2519x over previous
"""Trainium2 Bass kernel for nn_Agent_Aggregator_with_Mask_Denoise_Mechanism.

Sharding: tensor-parallel over heads h (8 heads -> 8 cores). Each core computes
its head end-to-end; the only cross-core value is the scalar threshold logit
(an 8-way AllReduce of a 4-byte partial sum). Host does the final (h d)
concat + transpose.

Math notes vs the reference:
- sigmoid(m) > sigmoid(t)  <=>  m > t: the mask threshold compares pre-sigmoid
  logits, so no sigmoid tables are needed (Exp only).
- The reference's kv.reshape(b, a, h*d) row-major remap means the thresh
  weight applied to kv[b,h,a,d] is w_thresh[0, (a%8)*64+d], independent of h.
- q is never materialized: qa logits = x @ (wq^T agent^T scale), the folded
  matrix precomputed on host, prescaled by 64, fp8 DoubleRow (2x rate);
  exp(logit/64) undoes the scale. k/v stay f16 (fp8 breaks the 2e-2 gate)
  but share one 128-wide stationary [v|k] so the PE array is fully used.
- Softmax normalizations fold into matmuls (ones columns / ones blocks);
  ka softmax skips max-subtraction (fp32 psum); fixed shift -14 fits fp16.

Perf notes (this device is DMA-packet-slot bound at ~87 packets/us, 4KB max
per packet, in addition to the PE):
- v^T -> v transposes run on the PE (transpose mode) instead of the DMA xbar:
  the xbar emits 256-B packets (1280 slots per block = most of the budget).
  The ones column of the kv stationary comes from memset-ing vsb to 1.0 once.
- x is loaded once in f16 (4KB packets) + once in fp8 pairs for the qa phase.
- The output is staged into one [128, 4096] SBUF tile per batch and shipped
  with a single large DMA.
- A dummy AllReduce at t=0 warms the collective path; the real AllReduce is
  issued right after the kv accumulation with the qa-denominator phase and
  the threshold-independent denoise epilogue executing under its latency.
- The threshold broadcast to 128 partitions is a [1,128]-ones matmul, not
  gpsimd partition_broadcast (measured ~15us there).
"""
import os
import sys

sys.path.insert(0, "/opt/trn_rl_repo")

import numpy as np
import ml_dtypes
from contextlib import ExitStack

import concourse.bass as bass
import concourse.tile as tile
from concourse import bacc, mybir, bass_utils

f32 = mybir.dt.float32
f16 = mybir.dt.float16
f8 = mybir.dt.float8e4

B, N, DIM = 2, 8192, 512
H, A, D = 8, 256, 64
N_CORES = 8
NBLK = 4            # 2048-column blocks per batch
SHIFT = -14.0       # ka exp shift to fit fp16
WS = 64.0           # fp8 agq prescale (undone via exp scale)

_cache = {}


def _install_profile_shim():
    """Restore the axon NTFF profile hook + disable artifact upload."""
    import contextlib
    import ctypes
    import types

    if "antenv.axon_hooks" in sys.modules:
        return
    so_path = "/opt/axon/libaxon_pjrt.so"
    holder = [None]
    mod = types.ModuleType("antenv.axon_hooks")
    mod.set_axon_ntff_profile_hook = lambda h: holder.__setitem__(0, h)
    mod.get_axon_ntff_profile_hook = lambda: holder[0]
    sys.modules["antenv.axon_hooks"] = mod
    try:
        lib = ctypes.CDLL(so_path)
        if hasattr(lib, "axon_start_nrt_profile"):
            lib.axon_start_nrt_profile.argtypes = [
                ctypes.POINTER(ctypes.c_int64),
                ctypes.c_size_t,
            ]
            lib.axon_start_nrt_profile.restype = ctypes.c_int64
            lib.axon_stop_nrt_profile.argtypes = [ctypes.c_char_p]
            lib.axon_stop_nrt_profile.restype = ctypes.c_int64

            @contextlib.contextmanager
            def _hook(output_dir, device_ids):
                import jax

                jax.devices()
                if device_ids:
                    ids = (ctypes.c_int64 * len(device_ids))(*device_ids)
                    rc = lib.axon_start_nrt_profile(ids, len(device_ids))
                else:
                    rc = lib.axon_start_nrt_profile(None, 0)
                if rc != 0:
                    raise RuntimeError(f"axon_start_nrt_profile rc={rc}")
                try:
                    yield
                finally:
                    n = lib.axon_stop_nrt_profile(str(output_dir).encode())
                    if n < 0:
                        raise RuntimeError(f"axon_stop_nrt_profile rc={n}")

            mod.set_axon_ntff_profile_hook(_hook)
    except OSError:
        pass
    bass_utils.upload_artifacts = lambda tmpdir: f"file://{tmpdir}"


def _build():
    nc = bacc.Bacc("TRN2", target_bir_lowering=False, debug=False,
                   num_devices=N_CORES)

    XT = nc.dram_tensor("xt", [B * 4, 128, N], f16, kind="ExternalInput").ap()
    XT8 = nc.dram_tensor("xt8", [B * 2, 128, 2, N], f8,
                         kind="ExternalInput").ap()
    WKV = nc.dram_tensor("wkv", [4, 128, 128], f16, kind="ExternalInput").ap()
    AGQ = nc.dram_tensor("agq", [4, 128, 2, 128], f8,
                         kind="ExternalInput").ap()
    AGS = nc.dram_tensor("ags", [128, 256], f16, kind="ExternalInput").ap()
    WN = nc.dram_tensor("wn", [64, 64], f32, kind="ExternalInput").ap()
    WM = nc.dram_tensor("wm", [64, 64], f32, kind="ExternalInput").ap()
    BN = nc.dram_tensor("bn", [128, 64], f32, kind="ExternalInput").ap()
    BM = nc.dram_tensor("bm", [128, 64], f32, kind="ExternalInput").ap()
    WTT = nc.dram_tensor("wtt", [2, 128, 64], f32, kind="ExternalInput").ap()
    IDENT = nc.dram_tensor("ident", [128, 128], f32, kind="ExternalInput").ap()
    ID16 = nc.dram_tensor("id16", [64, 64], f16, kind="ExternalInput").ap()
    BTHR = nc.dram_tensor("bthr", [128, 1], f32, kind="ExternalInput").ap()
    SONES8 = nc.dram_tensor("sones8", [2, 128, 2, 128], f8,
                            kind="ExternalInput").ap()
    OUT = nc.dram_tensor("out_t", [B, 128, 4096], f16,
                         kind="ExternalOutput").ap()

    EXP = mybir.ActivationFunctionType.Exp
    MULT = mybir.AluOpType.mult
    ADD = mybir.AluOpType.add
    GT = mybir.AluOpType.is_gt
    DR = mybir.MatmulPerfMode.DoubleRow
    INV_WS = 1.0 / WS

    with tile.TileContext(nc) as tc, ExitStack() as ctx:
        const = ctx.enter_context(tc.tile_pool(name="const", bufs=1))
        big = ctx.enter_context(tc.tile_pool(name="big", bufs=1))
        x8p = ctx.enter_context(tc.tile_pool(name="x8p", bufs=2))
        ek = ctx.enter_context(tc.tile_pool(name="ek", bufs=2))
        dsb = ctx.enter_context(tc.tile_pool(name="dsb", bufs=1))
        dram = ctx.enter_context(tc.tile_pool(name="dram", bufs=1, space="DRAM"))
        ps_vk = ctx.enter_context(tc.tile_pool(name="ps_vk", bufs=2, space="PSUM"))
        ps_lg = ctx.enter_context(tc.tile_pool(name="ps_lg", bufs=3, space="PSUM"))
        ps_kvt = ctx.enter_context(tc.tile_pool(name="ps_kvt", bufs=2, space="PSUM"))
        ps_sm = ctx.enter_context(tc.tile_pool(name="ps_sm", bufs=1, space="PSUM"))

        # ---- dummy AllReduce first: warms the collective path
        warm_sb = dsb.tile([1, 16], f32)
        nc.vector.memset(warm_sb[:], 0.0)
        wcc_in = dram.tile([1, 16], f32)
        wcc_out = dram.tile([1, 16], f32, addr_space="Shared")
        nc.sync.dma_start(wcc_in[:], warm_sb[:])
        nc.gpsimd.collective_compute(
            "AllReduce", ADD, ins=[wcc_in[:]], outs=[wcc_out[:]],
            replica_groups=[list(range(N_CORES))])

        # ---- constants to SBUF (gpsimd queue, ahead of the x8 loads)
        wkv_sb = []
        for dc in range(4):
            w1 = const.tile([128, 128], f16, name=f"wkv{dc}")
            nc.gpsimd.dma_start(w1[:], WKV[dc])
            wkv_sb.append(w1)
        agq_sb = []
        for g in range(4):
            w2 = const.tile([128, 2, 128], f8, name=f"agq{g}")
            nc.gpsimd.dma_start(w2[:, :, :], AGQ[g])
            agq_sb.append(w2)
        ags_sb = const.tile([128, 256], f16)
        nc.gpsimd.dma_start(ags_sb[:], AGS[:])
        wn_sb = const.tile([64, 64], f32)
        nc.gpsimd.dma_start(wn_sb[:], WN[:])
        wm_sb = const.tile([64, 64], f32)
        nc.gpsimd.dma_start(wm_sb[:], WM[:])
        bn_sb = const.tile([128, 64], f32)
        nc.gpsimd.dma_start(bn_sb[:], BN[:])
        bm_sb = const.tile([128, 64], f32)
        nc.gpsimd.dma_start(bm_sb[:], BM[:])
        wtt_sb = []
        for ac in range(2):
            w3 = const.tile([128, 64], f32, name=f"wtt{ac}")
            nc.gpsimd.dma_start(w3[:], WTT[ac])
            wtt_sb.append(w3)
        id_sb = const.tile([128, 128], f32)
        nc.gpsimd.dma_start(id_sb[:], IDENT[:])
        id16_sb = const.tile([64, 64], f16)
        nc.gpsimd.dma_start(id16_sb[:], ID16[:])
        bthr_sb = const.tile([128, 1], f32)
        nc.gpsimd.dma_start(bthr_sb[:], BTHR[:])
        bias_sh = const.tile([128, 1], f32)
        nc.vector.memset(bias_sh[:], SHIFT)
        sones8_sb = []
        for hf in range(2):
            s8 = const.tile([128, 2, 128], f8, name=f"sones8{hf}")
            nc.gpsimd.dma_start(s8[:, :, :], SONES8[hf])
            sones8_sb.append(s8)
        ones128 = nc.const_aps.tensor(1.0, [128, 1])
        ones_bc = nc.const_aps.tensor(1.0, [1, 128])

        # ---- persistent big tiles
        qkT = [big.tile([128, N], f16, name=f"qkT{b}") for b in range(B)]
        vsb = [big.tile([128, 64 * 65], f16, name=f"vsb{b}") for b in range(B)]
        vsb3 = [t[:].rearrange("p (c e) -> p c e", e=65) for t in vsb]
        for b in range(B):
            nc.vector.memset(vsb[b][:], 1.0)    # ones column for kv stat
        eqa8 = [big.tile([128, 2, N], f8, name=f"eqa{b}") for b in range(B)]

        # ---- x fp8 pair tiles for the qa phase (pre-issued, WAR-paced)
        x8_all = {}
        for blk in range(NBLK):
            for b in range(B):
                bsl = slice(blk * 2048, (blk + 1) * 2048)
                for p in range(2):
                    x8_t = x8p.tile([128, 2, 2048], f8,
                                    name=f"x8{blk}{b}{p}", tag=f"x8{b}{p}")
                    nc.gpsimd.dma_start(x8_t[:, :, :], XT8[b * 2 + p][:, :, bsl])
                    x8_all[blk, b, p] = x8_t

        # ---- phase-D/G state
        kvut = [None, None]
        zm_all = dsb.tile([128, 256], f32)    # mask logits * rs + bm
        den_all = dsb.tile([128, 256], f32)   # sigmoid denoise
        kv_all = dsb.tile([128, 256], f32)    # normalized kv
        rs_sb = {}
        r_sb = {}

        def emit_d(b, kvt_ps):
            t_u = dsb.tile([65, 256], f32, name=f"kvut{b}")
            nc.vector.tensor_copy(t_u[:], kvt_ps[b][:])
            kvut[b] = t_u
            for ac in range(2):
                g = b * 2 + ac
                gsl = slice(g * 64, (g + 1) * 64)
                asl = slice(ac * 128, (ac + 1) * 128)
                sm1 = ps_sm.tile([128, 65], f32, name="sm1", tag="sm")
                nc.tensor.matmul(sm1[:, 0:64], t_u[0:64, asl], wn_sb[:],
                                 start=True, stop=True)
                t_n = dsb.tile([128, 64], f32, name=f"noise{g}")
                nc.vector.tensor_copy(t_n[:], sm1[:, 0:64])
                sm2 = ps_sm.tile([128, 65], f32, name="sm2", tag="sm")
                nc.tensor.matmul(sm2[:, 0:64], t_u[0:64, asl], wm_sb[:],
                                 start=True, stop=True)
                t_m = dsb.tile([128, 64], f32, name=f"mask{g}")
                nc.vector.tensor_copy(t_m[:], sm2[:, 0:64])
                sm3 = ps_sm.tile([128, 65], f32, name="sm3", tag="sm")
                nc.tensor.transpose(sm3[:], t_u[:, asl], id_sb[0:65, 0:65])
                t_k = dsb.tile([128, 65], f32, name=f"kvn{g}")
                nc.vector.tensor_copy(t_k[:], sm3[:])
                t_rs = dsb.tile([128, 1], f32, name=f"rs{g}")
                nc.vector.reciprocal_approx_fast(t_rs[:], t_k[:, 64:65])
                rs_sb[b, ac] = t_rs
                nc.vector.tensor_scalar(out=kv_all[:, gsl], in0=t_k[:, 0:64],
                                        scalar1=t_rs[:], scalar2=None, op0=MULT)
                t_tmp = dsb.tile([128, 64], f32, name=f"tt{g}")
                nc.vector.tensor_tensor(t_tmp[:], kv_all[:, gsl],
                                        wtt_sb[ac][:], MULT)
                t_r = dsb.tile([128, 1], f32, name=f"r{g}")
                nc.vector.tensor_reduce(t_r[:], t_tmp[:],
                                        axis=mybir.AxisListType.X, op=ADD)
                r_sb[b, ac] = t_r
                # threshold-independent epilogue: mask pre-logit and denoise
                nc.vector.scalar_tensor_tensor(
                    out=zm_all[:, gsl], in0=t_m[:], scalar=t_rs[:],
                    in1=bm_sb[:], op0=MULT, op1=ADD)
                gn = dsb.tile([128, 64], f32, name=f"gn{g}")
                nc.vector.scalar_tensor_tensor(
                    out=gn[:], in0=t_n[:], scalar=t_rs[:],
                    in1=bn_sb[:], op0=MULT, op1=ADD)
                en = dsb.tile([128, 64], f32, name=f"en{g}")
                nc.scalar.activation(en[:], gn[:], EXP, scale=-1.0)
                dd = dsb.tile([128, 64], f32, name=f"dd{g}")
                nc.vector.tensor_scalar(out=dd[:], in0=en[:], scalar1=1.0,
                                        scalar2=None, op0=ADD)
                nc.vector.reciprocal_approx_fast(den_all[:, gsl], dd[:])

        def emit_f(blk, b):
            for sc in range(4):
                nck = blk * 4 + sc
                sl = slice(nck * 512, (nck + 1) * 512)
                ssl = slice(sc * 512, (sc + 1) * 512)
                for ac in range(2):
                    lgq = ps_lg.tile([128, 512], f32, name="lgq", tag="lg")
                    for p in range(2):
                        nc.tensor.matmul(
                            lgq[:], agq_sb[ac * 2 + p][:, :, :],
                            x8_all[blk, b, p][:, :, ssl],
                            start=(p == 0), stop=(p == 1), perf_mode=DR)
                    nc.scalar.activation(eqa8[b][:, ac, sl], lgq[:], EXP,
                                         scale=INV_WS)

        # == main loop: A ([v|k] proj) + T (PE transpose) + C (ka/kv) + F (qa)
        with ExitStack() as sA:
            xtp = sA.enter_context(tc.tile_pool(name="xtp", bufs=3))
            vtp = sA.enter_context(tc.tile_pool(name="vtp", bufs=2))
            kvt_ps = [ps_kvt.tile([65, 256], f32, name=f"kvtps{b}", tag="kvtps")
                      for b in range(B)]
            kv_mm_idx = [0, 0]
            # pre-issue x f16 loads; b0 on sync HWDGE, b1 on scalar HWDGE
            xts_all = {}
            for blk in range(NBLK):
                for b in range(B):
                    bsl = slice(blk * 2048, (blk + 1) * 2048)
                    for dc in range(4):
                        xt_t = xtp.tile([128, 2048], f16,
                                        name=f"x{blk}{b}{dc}", tag=f"x{dc}")
                        eng = nc.sync if b == 0 else nc.scalar
                        eng.dma_start(xt_t[:], XT[b * 4 + dc][:, bsl])
                        xts_all[blk, b, dc] = xt_t

            for blk in range(NBLK):
                for b in range(B):
                    xts = [xts_all[blk, b, dc] for dc in range(4)]
                    vt = vtp.tile([64, 2048], f16, name="vt", tag="vt")
                    # A: [v|k] projections for this 2048-col block
                    for sc in range(4):
                        nck = blk * 4 + sc
                        sl = slice(nck * 512, (nck + 1) * 512)
                        ssl = slice(sc * 512, (sc + 1) * 512)
                        vk_ps = ps_vk.tile([128, 512], f32, name="vkps",
                                           tag="vk")
                        for dc in range(4):
                            nc.tensor.matmul(vk_ps[:], wkv_sb[dc][:],
                                             xts[dc][:, ssl],
                                             start=(dc == 0), stop=(dc == 3))
                        nc.vector.tensor_copy(vt[:, ssl], vk_ps[0:64, :])
                        nc.vector.tensor_copy(qkT[b][64:128, sl],
                                              vk_ps[64:128, :])
                    # T: PE-mode transposes, 4 chunks per psum tile
                    for tg in range(4):
                        tr_ps = ps_vk.tile([128, 4, 64], f16, name="trps",
                                           tag="vk")
                        for j in range(4):
                            cc = tg * 4 + j
                            nc.tensor.transpose(
                                tr_ps[:, j, :], vt[:, cc * 128:(cc + 1) * 128],
                                id16_sb[:])
                        c0 = blk * 16 + tg * 4
                        nc.vector.tensor_copy(
                            vsb3[b][:, c0:c0 + 4, 0:64], tr_ps[:, :, :])
                    # C: ka logits -> exp -> kv^T accumulation
                    for cp in range(blk * 8, (blk + 1) * 8):
                        lg = ps_lg.tile([128, 512], f32, name="lg", tag="lg")
                        for j in range(2):
                            c = 2 * cp + j
                            nc.tensor.matmul(
                                lg[:, j * 256:(j + 1) * 256],
                                qkT[b][64:128, c * 128:(c + 1) * 128],
                                ags_sb[64:128, :],
                                start=True, stop=True)
                        e_t = ek.tile([128, 512], f16, name="eka", tag="eka")
                        nc.scalar.activation(e_t[:], lg[:], EXP,
                                             bias=bias_sh[:])
                        for j in range(2):
                            c = 2 * cp + j
                            ki = kv_mm_idx[b]
                            nc.tensor.matmul(
                                kvt_ps[b][:], vsb3[b][:, c, 0:65],
                                e_t[:, j * 256:(j + 1) * 256],
                                start=(ki == 0), stop=(ki == 63))
                            kv_mm_idx[b] += 1
                    # F: qa logits (fp8 DoubleRow) -> exp for this block.
                    # blk3 is deferred below the AllReduce to hide its latency
                    if blk < NBLK - 1:
                        emit_f(blk, b)
                    # D: per-batch epilogue right after its last C block
                    if blk == NBLK - 1:
                        emit_d(b, kvt_ps)

            # ---- threshold partial + the real AllReduce (phase S below
            # executes under its latency)
            th_ps = ps_sm.tile([1, 16], f32, name="thps", tag="sm")
            k = 0
            for b in range(B):
                for ac in range(2):
                    nc.tensor.matmul(th_ps[0:1, 0:1], r_sb[b, ac][:],
                                     ones128[0:128, :],
                                     start=(k == 0), stop=(k == 3))
                    k += 1
            th_sb = dsb.tile([1, 16], f32)
            nc.vector.memset(th_sb[:], 0.0)
            nc.vector.tensor_copy(th_sb[0:1, 0:1], th_ps[0:1, 0:1])
            cc_in = dram.tile([1, 16], f32)
            cc_out = dram.tile([1, 16], f32, addr_space="Shared")
            nc.sync.dma_start(cc_in[:], th_sb[:])
            nc.gpsimd.collective_compute(
                "AllReduce", ADD, ins=[cc_in[:]], outs=[cc_out[:]],
                replica_groups=[list(range(N_CORES))])

        # ===== deferred qa phase for the last block (under the collective)
        for b in range(B):
            emit_f(NBLK - 1, b)

        # ===== phase S: qa softmax denominators (under the collective) ======
        rsb = ctx.enter_context(tc.tile_pool(name="rsb", bufs=1))
        hsb = ctx.enter_context(tc.tile_pool(name="hsb", bufs=2))
        rso_sb = {}
        for b in range(B):
            for pr in range(8):
                sl0 = slice((2 * pr) * 512, (2 * pr + 1) * 512)
                sl1 = slice((2 * pr + 1) * 512, (2 * pr + 2) * 512)
                s_ps = ps_vk.tile([128, 512], f32, name="sps", tag="vk")
                nc.tensor.matmul(s_ps[:], sones8_sb[0][:, :, :],
                                 eqa8[b][:, :, sl0], start=True, stop=False,
                                 perf_mode=DR)
                nc.tensor.matmul(s_ps[:], sones8_sb[1][:, :, :],
                                 eqa8[b][:, :, sl1], start=False, stop=True,
                                 perf_mode=DR)
                rtmp = hsb.tile([128, 512], f32, name="rtmp", tag="rtmp")
                nc.vector.reciprocal_approx_fast(rtmp[:], s_ps[:])
                rso = rsb.tile([128, 512], f16, name=f"rso{b}{pr}")
                nc.vector.tensor_copy(rso[:], rtmp[:])
                rso_sb[b, pr] = rso

        # ---- collective result -> threshold scalar (PE broadcast)
        ts_sb = dsb.tile([1, 16], f32)
        nc.sync.dma_start(ts_sb[:], cc_out[:])
        tb_ps = ps_sm.tile([128, 16], f32, name="tbps", tag="sm")
        nc.tensor.matmul(tb_ps[:, 0:1], ones_bc[:], ts_sb[0:1, 0:1],
                         start=True, stop=True)
        tfin = dsb.tile([128, 1], f32)
        nc.vector.tensor_scalar(out=tfin[:], in0=tb_ps[:, 0:1],
                                scalar1=1.0 / (B * A), scalar2=bthr_sb[:],
                                op0=MULT, op1=ADD)

        # ========== phase G: thresholded mask + second softmax ==============
        mb = dsb.tile([128, 256], f32)
        nc.vector.tensor_scalar(out=mb[:], in0=zm_all[:], scalar1=tfin[:],
                                scalar2=None, op0=GT)
        kvm = dsb.tile([128, 256], f32)
        nc.vector.tensor_tensor(kvm[:], kv_all[:], mb[:], MULT)
        l2 = dsb.tile([128, 256], f32)
        nc.vector.tensor_tensor(l2[:], kvm[:], den_all[:], ADD)
        e2 = dsb.tile([128, 256], f32)
        s24 = dsb.tile([128, 4], f32)
        for g in range(4):
            gsl = slice(g * 64, (g + 1) * 64)
            nc.scalar.activation(e2[:, gsl], l2[:, gsl], EXP,
                                 accum_out=s24[:, g:g + 1])
        rs24 = dsb.tile([128, 4], f32)
        nc.vector.reciprocal_approx_fast(rs24[:], s24[:])
        kv28z = [dsb.tile([128, 2, 128], f8, name=f"kv28z{b}{hf}")
                 for b in range(B) for hf in range(2)]
        for t in kv28z:
            nc.vector.memset(t[:, :, :], 0.0)
        for g in range(4):
            b, ac = divmod(g, 2)
            gsl = slice(g * 64, (g + 1) * 64)
            for hf in range(2):
                nc.vector.tensor_scalar(
                    out=kv28z[b * 2 + hf][:, ac, hf * 64:(hf + 1) * 64],
                    in0=e2[:, gsl], scalar1=rs24[:, g:g + 1], scalar2=64.0,
                    op0=MULT, op1=MULT)

        # ===== phase H: out^T = kv2^T @ E_qa^T, staged output ===============
        ostg = [rsb.tile([128, 4096], f16, name=f"ostg{b}") for b in range(B)]
        for b in range(B):
            for pr in range(8):
                sl0 = slice((2 * pr) * 512, (2 * pr + 1) * 512)
                sl1 = slice((2 * pr + 1) * 512, (2 * pr + 2) * 512)
                pool = ps_lg if pr % 2 == 0 else ps_vk
                tg2 = "lg" if pr % 2 == 0 else "vk"
                o_ps = pool.tile([128, 512], f32, name="ops", tag=tg2)
                nc.tensor.matmul(o_ps[:], kv28z[b * 2][:, :, :],
                                 eqa8[b][:, :, sl0], start=True, stop=False,
                                 perf_mode=DR)
                nc.tensor.matmul(o_ps[:], kv28z[b * 2 + 1][:, :, :],
                                 eqa8[b][:, :, sl1], start=False, stop=True,
                                 perf_mode=DR)
                nc.vector.tensor_tensor(ostg[b][:, pr * 512:(pr + 1) * 512],
                                        o_ps[:], rso_sb[b, pr][:], MULT)
            nc.scalar.dma_start(OUT[b], ostg[b][:])

    nc.compile()
    return nc


def _prep_inputs(x, w_qkv, agent, w_noise, b_noise, w_mask, b_mask,
                 w_thresh, b_thresh):
    scale = D ** -0.5
    xt = np.ascontiguousarray(
        x.transpose(0, 2, 1).astype(np.float16)).reshape(B * 4, 128, N)
    xt8 = np.ascontiguousarray(
        x.transpose(0, 2, 1).reshape(B, 2, 2, 128, N).transpose(0, 1, 3, 2, 4)
        .reshape(B * 2, 128, 2, N)).astype(ml_dtypes.float8_e4m3)
    wq = w_qkv[0:H * D].reshape(H, D, DIM)
    wk = w_qkv[H * D:2 * H * D].reshape(H, D, DIM)
    wv = w_qkv[2 * H * D:3 * H * D].reshape(H, D, DIM)
    bn_rep = np.ascontiguousarray(
        np.broadcast_to(b_noise[None, :], (128, 64))).astype(np.float32)
    bm_rep = np.ascontiguousarray(
        np.broadcast_to(b_mask[None, :], (128, 64))).astype(np.float32)
    wtt = np.zeros((A, D), np.float32)
    for a in range(A):
        wtt[a] = w_thresh[0, (a % 8) * D:(a % 8 + 1) * D]
    wtt = wtt.reshape(2, 128, 64)
    ident = np.eye(128, dtype=np.float32)
    sones8 = np.zeros((2, 128, 2, 128), dtype=np.float32)
    sones8[0, :, :, 0:64] = WS
    sones8[1, :, :, 64:128] = WS
    sones8 = sones8.astype(ml_dtypes.float8_e4m3)
    id16 = np.eye(64, dtype=np.float16)
    bthr = np.full((128, 1), float(np.asarray(b_thresh).ravel()[0]), np.float32)
    in_maps = []
    for h in range(H):
        wvk_h = np.concatenate([wv[h], wk[h]], axis=0)            # [128, 512]
        wvk_t = np.ascontiguousarray(wvk_h.T).astype(np.float16)  # [512, 128]
        agq = np.einsum('dc,ad->ca', wq[h].astype(np.float64),
                        agent[h].astype(np.float64)) * (scale * WS)
        agq = agq.astype(np.float32)                              # [DIM, A]
        agq4 = np.empty((4, 128, 2, 128), np.float32)
        for ac in range(2):
            for p in range(2):
                for pl in range(2):
                    rows = slice(p * 256 + pl * 128, p * 256 + (pl + 1) * 128)
                    agq4[ac * 2 + p, :, pl, :] = \
                        agq[rows, ac * 128:(ac + 1) * 128]
        agq4 = agq4.astype(ml_dtypes.float8_e4m3)
        ags = np.ascontiguousarray(np.concatenate(
            [agent[h].T, agent[h].T], axis=0)).astype(np.float16)
        in_maps.append({
            "xt": xt,
            "xt8": xt8,
            "wkv": np.ascontiguousarray(wvk_t.reshape(4, 128, 128)),
            "agq": agq4,
            "ags": ags,
            "wn": np.ascontiguousarray(w_noise.T).astype(np.float32),
            "wm": np.ascontiguousarray(w_mask.T).astype(np.float32),
            "bn": bn_rep,
            "bm": bm_rep,
            "wtt": wtt,
            "ident": ident,
            "id16": id16,
            "bthr": bthr,
            "sones8": sones8,
        })
    return in_maps


LAST_EXEC_NS = None
LAST_RES = None


def kernel(**inputs):
    global LAST_EXEC_NS, LAST_RES
    _install_profile_shim()
    if "nc" not in _cache:
        _cache["nc"] = _build()
    nc = _cache["nc"]
    inputs = {k: np.asarray(v) for k, v in inputs.items()}
    in_maps = _prep_inputs(**inputs)
    trace = os.environ.get("BASS_KERNEL_TRACE", "0") == "1"
    res = bass_utils.run_bass_kernel_spmd(
        nc, in_maps, core_ids=list(range(N_CORES)), trace=trace)
    LAST_EXEC_NS = res.exec_time_ns
    LAST_RES = res
    out = np.empty((B, N, H * D), np.float32)
    for h in range(H):
        o = np.asarray(res.results[h]["out_t"]).astype(np.float32)
        # o[b, (half,d), (pr,c)]: n = pr*1024 + half*512 + c
        o2 = o.reshape(B, 2, 64, 8, 512)         # [b, half, d, pr, c]
        o3 = o2.transpose(0, 3, 1, 4, 2).reshape(B, N, D)
        out[:, :, h * D:(h + 1) * D] = o3
    return out
